# revision 7
# baseline (speedup 1.0000x reference)
"""Trainium2 Bass kernel for nn_MelToWaveform (InverseMelScale + GriffinLim).

Strategy: batch=8 data-parallel across 8 NeuronCores (one sample per core).
All FFTs are expressed as fp32 PE matmuls against precomputed DFT matrices in a
stacked-real layout (1024 = 513 Re + 511 Im). The Griffin-Lim overlap-add is
fused into the ISTFT matmuls via PSUM accumulation of 4 hop-shifted matmul
groups; STFT framing is pure AP-shifted views of the on-chip signal matrix.
Reflect-padding edges are reproduced with tiny permutation matmuls.

Everything stays in SBUF/PSUM for all 32 iterations; DMA only at start/end.
"""
import numpy as np
from contextlib import ExitStack

import jax

try:  # need CPU for pinv/PRNG while axon keeps the neuron cores
    _p = jax.config.jax_platforms
    if _p and "cpu" not in _p:
        jax.config.update("jax_platforms", _p + ",cpu")
except Exception:
    pass
import jax.numpy as jnp

import concourse.bass as bass
import concourse.tile as tile
from concourse import bacc, mybir
from concourse import bass_utils

F32 = mybir.dt.float32
AF = mybir.ActivationFunctionType
ALU = mybir.AluOpType

N_FFT = 1024
HOP = 256
T = 1024
NF = 513
PAD = 512
N_ITER = 32
L = N_FFT + HOP * (T - 1)
MOM = float(np.float32(0.99 / 1.99))

# edge reflection table: (psum_col, h, [(flip_idx, src_h, src_col), ...])
# psum cols 0..3 correspond to Y columns u = 0, 1, 1025, 1026
EDGE_SPECS = [
    (0, 0, [(0, 1, 3), (2, 0, 4)]),
    (0, 1, [(0, 0, 3), (2, 1, 3)]),
    (1, 0, [(0, 1, 2), (2, 0, 3)]),
    (1, 1, [(0, 0, 2), (2, 1, 2)]),
    (2, 0, [(1, 1, 1024), (3, 0, 1024)]),
    (2, 1, [(1, 0, 1024), (3, 1, 1023)]),
    (3, 0, [(1, 1, 1023), (3, 0, 1023)]),
    (3, 1, [(1, 0, 1023), (3, 1, 1022)]),
]
UCOL = [0, 1, 1025, 1026]

_consts_cache = None
_prog_cache = {}


def _constants():
    global _consts_cache
    if _consts_cache is not None:
        return _consts_cache
    WIN = 0.5 * (1.0 - np.cos(2.0 * np.pi * np.arange(N_FFT) / N_FFT))
    n = np.arange(N_FFT)
    k = np.arange(NF)
    C = np.cos(2 * np.pi * np.outer(n, k) / N_FFT) / N_FFT
    C[:, 1:512] *= 2.0
    S = -2.0 * np.sin(2 * np.pi * np.outer(n, np.arange(1, 512)) / N_FFT) / N_FFT
    R = np.concatenate([C, S], axis=1)          # (1024 samples, 1024 stacked)
    Rw = WIN[:, None] * R
    Wc = np.cos(2 * np.pi * np.outer(k, n) / N_FFT)
    Ws = -np.sin(2 * np.pi * np.outer(np.arange(1, 512), n) / N_FFT)
    Wm = np.concatenate([Wc, Ws], axis=0) * WIN[None, :]  # (1024 stacked, 1024 t)

    idx = np.arange(T)[:, None] * HOP + np.arange(N_FFT)
    wsq = np.zeros(L)
    np.add.at(wsq, idx.reshape(-1), np.tile(WIN ** 2, T))
    wsq_t = wsq[PAD:L - PAD]
    rw = 1.0 / np.where(wsq_t > 1e-11, wsq_t, 1.0)   # (261888,)
    rwm = rw.reshape(1023, 256).T.copy()              # (256, 1023)

    flips = np.zeros((4, 128, 128), np.float32)
    flips[0][np.arange(1, 128), 128 - np.arange(1, 128)] = 1  # FLIP128
    flips[1][np.arange(0, 127), 126 - np.arange(0, 127)] = 1  # FLIP126
    flips[2][0, 0] = 1                                         # S00
    flips[3][127, 127] = 1                                     # S127

    _consts_cache = dict(
        rwt=np.ascontiguousarray(Rw.T, np.float32),   # (k stacked, m sample)
        wt=np.ascontiguousarray(Wm.T, np.float32),    # (k time, m stacked)
        rwm=np.ascontiguousarray(rwm, np.float32),
        flips=flips,
    )
    return _consts_cache


def _build_program(n_iter=N_ITER):
    """Build + compile the per-core Bass program. Cached per n_iter."""
    if n_iter in _prog_cache:
        return _prog_cache[n_iter]

    nc = bacc.Bacc("TRN2", target_bir_lowering=False, debug=False, num_devices=8)

    mel_d = nc.dram_tensor("mel", [80, 1024], F32, kind="ExternalInput").ap()
    ang_d = nc.dram_tensor("ang", [1024, 1024], F32, kind="ExternalInput").ap()
    pt_d = nc.dram_tensor("pt", [80, 513], F32, kind="ExternalInput").ap()
    rwt_d = nc.dram_tensor("rwt", [1024, 1024], F32, kind="ExternalInput").ap()
    wt_d = nc.dram_tensor("wt", [1024, 1024], F32, kind="ExternalInput").ap()
    rwm0_d = nc.dram_tensor("rwm0", [128, 1023], F32, kind="ExternalInput").ap()
    rwm1_d = nc.dram_tensor("rwm1", [128, 1023], F32, kind="ExternalInput").ap()
    fl_d = [nc.dram_tensor(f"fl{i}", [128, 128], F32, kind="ExternalInput").ap()
            for i in range(4)]
    y0_d = nc.dram_tensor("y0", [128, 1023], F32, kind="ExternalOutput").ap()
    y1_d = nc.dram_tensor("y1", [128, 1023], F32, kind="ExternalOutput").ap()

    with tile.TileContext(nc) as tc, ExitStack() as ctx:
        cpool = ctx.enter_context(tc.tile_pool(name="const", bufs=1))
        mpool = ctx.enter_context(tc.tile_pool(name="mag", bufs=1))
        vpool = ctx.enter_context(tc.tile_pool(name="v", bufs=1))
        tpool = ctx.enter_context(tc.tile_pool(name="tp", bufs=1))
        ypool = ctx.enter_context(tc.tile_pool(name="y", bufs=1))
        spool = ctx.enter_context(tc.tile_pool(name="s", bufs=1))
        upool = ctx.enter_context(tc.tile_pool(name="u", bufs=1))
        papool = ctx.enter_context(tc.tile_pool(name="pa", bufs=2, space="PSUM"))
        pbpool = ctx.enter_context(tc.tile_pool(name="pb", bufs=2, space="PSUM"))

        # ---- constants into SBUF ----
        rwt_t = []
        wt_t = []
        for kc in range(8):
            t_ = cpool.tile([128, 1024], F32, tag=f"rwt{kc}", name=f"rwt{kc}")
            nc.sync.dma_start(t_[:], rwt_d[128 * kc:128 * (kc + 1), :])
            rwt_t.append(t_)
        for kc in range(8):
            t_ = cpool.tile([128, 1024], F32, tag=f"wt{kc}", name=f"wt{kc}")
            nc.sync.dma_start(t_[:], wt_d[128 * kc:128 * (kc + 1), :])
            wt_t.append(t_)
        pt_t = cpool.tile([80, 513], F32, tag="pt", name="pt")
        nc.sync.dma_start(pt_t[:], pt_d[:])
        rwm_t = []
        for h, d in enumerate([rwm0_d, rwm1_d]):
            t_ = cpool.tile([128, 1023], F32, tag=f"rwm{h}", name=f"rwm{h}")
            nc.sync.dma_start(t_[:], d[:])
            rwm_t.append(t_)
        fl_t = []
        for i in range(4):
            t_ = cpool.tile([128, 128], F32, tag=f"fl{i}", name=f"fl{i}")
            nc.sync.dma_start(t_[:], fl_d[i][:])
            fl_t.append(t_)
        bias_t = cpool.tile([128, 1], F32, tag="bias", name="bias")
        nc.vector.memset(bias_t[:], 1e-32)
        mel_t = cpool.tile([80, 1024], F32, tag="mel", name="mel")
        nc.sync.dma_start(mel_t[:], mel_d[:])

        # ---- V tiles (padded: cols 3..1026 hold frame data) ----
        v_t = []
        for c in range(8):
            t_ = vpool.tile([128, 1030], F32, tag=f"v{c}", name=f"v{c}")
            nc.vector.memset(t_[:, 0:3], 0.0)
            nc.vector.memset(t_[:, 1027:1030], 0.0)
            nc.sync.dma_start(t_[:, 3:1027], ang_d[128 * c:128 * (c + 1), :])
            v_t.append(t_)

        # ---- spec = relu(P @ mel); mag = sqrt(spec) with one Newton step ----
        m_t = [mpool.tile([128, 1024], F32, tag=f"m{c}", name=f"m{c}") for c in range(4)]
        m4_t = mpool.tile([128, 1024], F32, tag="m4", name="m4")
        def newton_mag(dst, M):
            q = spool.tile([128, 1024], F32, tag="s", name="s")
            r = spool.tile([128, 1024], F32, tag="t1", name="t1")
            nc.scalar.activation(q[0:M, :], dst[0:M, :], AF.Sqrt, bias=bias_t[0:M])
            nc.vector.reciprocal(r[0:M, :], q[0:M, :])
            nc.vector.scalar_tensor_tensor(
                dst[0:M, :], dst[0:M, :], 0.5, r[0:M, :], ALU.mult, ALU.mult)
            nc.vector.scalar_tensor_tensor(
                dst[0:M, :], q[0:M, :], 0.5, dst[0:M, :], ALU.mult, ALU.add)

        for mc in range(4):
            for half in range(2):
                ps = papool.tile([128, 512], F32, tag="a0", name="a0")
                nc.tensor.matmul(ps[:], pt_t[:, 128 * mc:128 * (mc + 1)],
                                 mel_t[:, 512 * half:512 * (half + 1)],
                                 start=True, stop=True)
                nc.vector.tensor_scalar_max(
                    m_t[mc][:, 512 * half:512 * (half + 1)], ps[:], 0.0)
            newton_mag(m_t[mc], 128)
        # stacked-mag chunk 4: rows 1..127 hold mag bins 1..127 (copied from
        # chunk 0); row 0 = mag bin 512, computed after the copy overwrites it.
        nc.vector.tensor_copy(m4_t[:], m_t[0][:])
        for half in range(2):
            ps = papool.tile([128, 512], F32, tag="a0", name="a0")
            nc.tensor.matmul(ps[0:1, :], pt_t[:, 512:513],
                             mel_t[:, 512 * half:512 * (half + 1)],
                             start=True, stop=True)
            nc.vector.tensor_scalar_max(
                m4_t[0:1, 512 * half:512 * (half + 1)], ps[0:1, :], 0.0)
        newton_mag(m4_t, 1)
        mag_st = [m_t[0], m_t[1], m_t[2], m_t[3], m4_t, m_t[1], m_t[2], m_t[3]]

        # ---- V0 = mag_st * ang ----
        for c in range(8):
            nc.vector.tensor_mul(v_t[c][:, 3:1027], v_t[c][:, 3:1027], mag_st[c][:])

        # ---- tprev = 0 ----
        tp_t = []
        for c in range(8):
            t_ = tpool.tile([128, 1024], F32, tag=f"tp{c}", name=f"tp{c}")
            nc.vector.memset(t_[:], 0.0)
            tp_t.append(t_)

        y_t = [ypool.tile([128, 1027], F32, tag=f"y{h}", name=f"y{h}") for h in range(2)]
        # patch scratch, single partition: cols 0:1024 = bin-0 result,
        # 1024:2048 = bin-512 result, 2048:3072 = workspace
        pr_t = upool.tile([1, 3072], F32, tag="patch", name="patch")
        prA = pr_t[0:1, 0:1024]
        prB = pr_t[0:1, 1024:2048]
        prC = pr_t[0:1, 2048:3072]

        # ---- main loop ----
        for it in range(n_iter + 1):
            last = it == n_iter
            # Phase A: ISTFT matmuls with fused OLA, then envelope multiply
            for h in range(2):
                pa0 = papool.tile([128, 512], F32, tag="a0", name="a0")
                pa12 = papool.tile([128, 515], F32, tag="a12", name="a12")
                for a in range(4):
                    for kc in range(8):
                        lhsT = rwt_t[kc][:, 256 * a + 128 * h:256 * a + 128 * h + 128]
                        st = (a == 0 and kc == 0)
                        sp = (a == 3 and kc == 7)
                        nc.tensor.matmul(pa0[:, 0:512], lhsT,
                                         v_t[kc][:, 3 - a:515 - a],
                                         start=st, stop=sp)
                        nc.tensor.matmul(pa12[:, 0:512], lhsT,
                                         v_t[kc][:, 515 - a:1027 - a],
                                         start=st, stop=sp)
                        nc.tensor.matmul(pa12[:, 512:515], lhsT,
                                         v_t[kc][:, 1027 - a:1030 - a],
                                         start=st, stop=sp)
                nc.vector.tensor_mul(y_t[h][:, 2:512], pa0[:, 2:512],
                                     rwm_t[h][:, 0:510])
                nc.vector.tensor_mul(y_t[h][:, 512:1025], pa12[:, 0:513],
                                     rwm_t[h][:, 510:1023])
            if last:
                nc.sync.dma_start(y0_d[:], y_t[0][:, 2:1025])
                nc.sync.dma_start(y1_d[:], y_t[1][:, 2:1025])
                break

            # reflect-pad edge columns via permutation matmuls
            for h in range(2):
                pe_ = pbpool.tile([128, 4], F32, tag="b", name="b")
                for col, hh, specs in EDGE_SPECS:
                    if hh != h:
                        continue
                    for i2, (fi, sh, sc) in enumerate(specs):
                        nc.tensor.matmul(pe_[:, col:col + 1], fl_t[fi][:],
                                         y_t[sh][:, sc:sc + 1],
                                         start=(i2 == 0), stop=(i2 == len(specs) - 1))
                nc.vector.tensor_copy(y_t[h][:, 0:2], pe_[:, 0:2])
                nc.vector.tensor_copy(y_t[h][:, 1025:1027], pe_[:, 2:4])

            # Phase B: STFT matmuls (framing = shifted views of Y), then C:
            # new = rebuilt - mom*tprev; tprev = rebuilt; V = mag * new/|new|
            for mc in [0, 4, 1, 5, 2, 6, 3, 7]:
                for half in range(2):
                    pb = pbpool.tile([128, 512], F32, tag="b", name="b")
                    for kc in range(8):
                        ap_, hh = divmod(kc, 2)
                        nc.tensor.matmul(
                            pb[:], wt_t[kc][:, 128 * mc:128 * (mc + 1)],
                            y_t[hh][:, ap_ + 512 * half:ap_ + 512 * half + 512],
                            start=(kc == 0), stop=(kc == 7))
                    nc.vector.scalar_tensor_tensor(
                        v_t[mc][:, 3 + 512 * half:3 + 512 * (half + 1)],
                        tp_t[mc][:, 512 * half:512 * (half + 1)],
                        -MOM, pb[:], ALU.mult, ALU.add)
                    nc.scalar.copy(tp_t[mc][:, 512 * half:512 * (half + 1)], pb[:])
                if mc >= 4:
                    c = mc - 4
                    vc = v_t[c][:, 3:1027]
                    vc4 = v_t[c + 4][:, 3:1027]
                    s = spool.tile([128, 1024], F32, tag="s", name="s")
                    t1 = spool.tile([128, 1024], F32, tag="t1", name="t1")
                    nc.vector.tensor_mul(s[:], vc, vc)
                    nc.vector.tensor_mul(t1[:], vc4, vc4)
                    nc.vector.tensor_add(s[:], s[:], t1[:])
                    nc.scalar.activation(t1[:], s[:], AF.Sqrt, bias=bias_t[:])
                    nc.vector.reciprocal(s[:], t1[:])
                    nc.vector.tensor_mul(s[:], s[:], mag_st[c][:])
                    if c == 0:
                        # bins 0 and 512 are real-valued pairs sharing a
                        # partition: normalize each by its own |x| instead
                        v0r = v_t[0][0:1, 3:1027]
                        v4r = v_t[4][0:1, 3:1027]
                        nc.vector.tensor_mul(prC, v0r, v0r)
                        nc.scalar.activation(prA, prC, AF.Sqrt, bias=bias_t[0:1])
                        nc.vector.reciprocal(prC, prA)
                        nc.vector.tensor_mul(prC, prC, m_t[0][0:1, :])
                        nc.vector.tensor_mul(prA, prC, v0r)
                        nc.vector.tensor_mul(prC, v4r, v4r)
                        nc.scalar.activation(prB, prC, AF.Sqrt, bias=bias_t[0:1])
                        nc.vector.reciprocal(prC, prB)
                        nc.vector.tensor_mul(prC, prC, m4_t[0:1, :])
                        nc.vector.tensor_mul(prB, prC, v4r)
                    nc.vector.tensor_mul(vc, vc, s[:])
                    nc.vector.tensor_mul(vc4, vc4, s[:])
                    if c == 0:
                        nc.vector.tensor_copy(v_t[0][0:1, 3:1027], prA)
                        nc.vector.tensor_copy(v_t[4][0:1, 3:1027], prB)

    nc.compile()
    _prog_cache[n_iter] = nc
    return nc


def _host_inputs(mel_spec, fb):
    """Per-core input maps (host-side prep: pinv + initial random phases)."""
    consts = _constants()
    cpu = jax.devices("cpu")[0]
    with jax.default_device(cpu):
        P = np.asarray(jnp.linalg.pinv(jnp.asarray(np.asarray(fb)).T))  # (513, 80)
        k1, k2 = jax.random.split(jax.random.key(42))
        sh = (8, NF, T)
        ar = np.asarray(jax.random.uniform(k1, sh, jnp.float32))
        ai = np.asarray(jax.random.uniform(k2, sh, jnp.float32))
    pt = np.ascontiguousarray(P.T, np.float32)  # (80, 513)
    mel = np.asarray(mel_spec, np.float32)
    in_maps = []
    for b in range(8):
        ang = np.concatenate([ar[b], ai[b][1:512]], axis=0)  # (1024, 1024)
        in_maps.append(dict(
            mel=np.ascontiguousarray(mel[b]),
            ang=np.ascontiguousarray(ang, np.float32),
            pt=pt,
            rwt=consts["rwt"],
            wt=consts["wt"],
            rwm0=np.ascontiguousarray(consts["rwm"][0:128]),
            rwm1=np.ascontiguousarray(consts["rwm"][128:256]),
            fl0=consts["flips"][0], fl1=consts["flips"][1],
            fl2=consts["flips"][2], fl3=consts["flips"][3],
        ))
    return in_maps


def _assemble(results):
    out = np.empty((8, L - 2 * PAD), np.float32)
    for b, r in enumerate(results):
        yfull = np.concatenate([r["y0"], r["y1"]], axis=0)  # (256, 1023)
        out[b] = yfull.T.reshape(-1)
    return out


def kernel(mel_spec, fb):
    nc = _build_program(N_ITER)
    in_maps = _host_inputs(mel_spec, fb)
    res = bass_utils.run_bass_kernel_spmd(nc, in_maps, core_ids=list(range(8)))
    return _assemble(res.results)


# revision 8
# speedup vs baseline: 7.4976x; 7.4976x over previous
"""Trainium2 Bass kernel for nn_MelToWaveform — STFT-folded variant.

Same as kernel.py but the STFT contraction is halved via even/odd folding:
frames e[n] = f[n]+f[1024-n], o[n] = f[n]-f[1024-n] reduce the rfft matmul
contract from 1024 to 512 (cos rows consume e, sin rows consume o). The
partition reversal needed for the fold is done with tiny PE permutation
matmuls into PSUM; e/o are materialized per 512-column half to fit SBUF.
"""
import numpy as np
from contextlib import ExitStack

import jax

try:
    _p = jax.config.jax_platforms
    if _p and "cpu" not in _p:
        jax.config.update("jax_platforms", _p + ",cpu")
except Exception:
    pass
import jax.numpy as jnp

import concourse.bass as bass
import concourse.tile as tile
from concourse import bacc, mybir
from concourse import bass_utils

F32 = mybir.dt.float32
AF = mybir.ActivationFunctionType
ALU = mybir.AluOpType

N_FFT = 1024
HOP = 256
T = 1024
NF = 513
PAD = 512
N_ITER = 32
L = N_FFT + HOP * (T - 1)
MOM = float(np.float32(0.99 / 1.99))
C23 = float(np.float32(1.0) / np.float32(1.5))

EDGE_SPECS = [
    (0, 0, [(0, 1, 3), (2, 0, 4)]),
    (0, 1, [(0, 0, 3), (2, 1, 3)]),
    (1, 0, [(0, 1, 2), (2, 0, 3)]),
    (1, 1, [(0, 0, 2), (2, 1, 2)]),
    (2, 0, [(1, 1, 1024), (3, 0, 1024)]),
    (2, 1, [(1, 0, 1024), (3, 1, 1023)]),
    (3, 0, [(1, 1, 1023), (3, 0, 1023)]),
    (3, 1, [(1, 0, 1023), (3, 1, 1022)]),
]
UCOL = [0, 1, 1025, 1026]

_consts_cache = None
_prog_cache = {}


def _constants():
    global _consts_cache
    if _consts_cache is not None:
        return _consts_cache
    WIN = 0.5 * (1.0 - np.cos(2.0 * np.pi * np.arange(N_FFT) / N_FFT))
    n = np.arange(N_FFT)
    k = np.arange(NF)
    C = np.cos(2 * np.pi * np.outer(n, k) / N_FFT) / N_FFT
    C[:, 1:512] *= 2.0
    S = -2.0 * np.sin(2 * np.pi * np.outer(n, np.arange(1, 512)) / N_FFT) / N_FFT
    R = np.concatenate([C, S], axis=1)
    Rw = WIN[:, None] * R

    # folded stft weights: contract n' = 0..511 (n'=0 carries frame[512])
    kk = np.arange(513)
    nn = np.arange(512)
    WE = np.zeros((1024, 512))
    WE[0:513, :] = WIN[None, nn] * np.cos(2 * np.pi * np.outer(kk, nn) / N_FFT)
    WE[0:513, 0] = WIN[512] * np.cos(np.pi * kk)
    kim = np.arange(1, 512)
    WO = np.zeros((1024, 512))
    WO[513:1024, :] = -WIN[None, nn] * np.sin(2 * np.pi * np.outer(kim, nn) / N_FFT)
    WO[513:1024, 0] = 0.0

    idx = np.arange(T)[:, None] * HOP + np.arange(N_FFT)
    wsq = np.zeros(L)
    np.add.at(wsq, idx.reshape(-1), np.tile(WIN ** 2, T))
    wsq_t = wsq[PAD:L - PAD]
    rw = 1.0 / np.where(wsq_t > 1e-11, wsq_t, 1.0)
    rwm_full = rw.reshape(1023, 256).T  # (256, 1023)
    # envelope is exactly 2/3 everywhere except columns j=0 and j=1022
    rwm = np.stack([rwm_full[:, 0], rwm_full[:, 1022]], axis=1)  # (256, 2)

    flips = np.zeros((4, 128, 128), np.float32)
    flips[0][np.arange(1, 128), 128 - np.arange(1, 128)] = 1
    flips[1][np.arange(0, 127), 126 - np.arange(0, 127)] = 1
    flips[2][0, 0] = 1
    flips[3][127, 127] = 1

    _consts_cache = dict(
        rwt=np.ascontiguousarray(Rw.T, np.float32),
        wet=np.ascontiguousarray(WE.T, np.float32),   # (512 contract, 1024 out)
        wot=np.ascontiguousarray(WO.T, np.float32),
        rwm=np.ascontiguousarray(rwm, np.float32),
        flips=flips,
    )
    return _consts_cache


def _build_program(n_iter=N_ITER):
    if n_iter in _prog_cache:
        return _prog_cache[n_iter]

    nc = bacc.Bacc("TRN2", target_bir_lowering=False, debug=False, num_devices=8)

    mel_d = nc.dram_tensor("mel", [80, 1024], F32, kind="ExternalInput").ap()
    ang_d = nc.dram_tensor("ang", [1024, 1024], F32, kind="ExternalInput").ap()
    pt_d = nc.dram_tensor("pt", [80, 513], F32, kind="ExternalInput").ap()
    rwt_d = nc.dram_tensor("rwt", [1024, 1024], F32, kind="ExternalInput").ap()
    wet_d = nc.dram_tensor("wet", [512, 1024], F32, kind="ExternalInput").ap()
    wot_d = nc.dram_tensor("wot", [512, 1024], F32, kind="ExternalInput").ap()
    rwm0_d = nc.dram_tensor("rwm0", [128, 2], F32, kind="ExternalInput").ap()
    rwm1_d = nc.dram_tensor("rwm1", [128, 2], F32, kind="ExternalInput").ap()
    fl_d = [nc.dram_tensor(f"fl{i}", [128, 128], F32, kind="ExternalInput").ap()
            for i in range(4)]
    y0_d = nc.dram_tensor("y0", [128, 1023], F32, kind="ExternalOutput").ap()
    y1_d = nc.dram_tensor("y1", [128, 1023], F32, kind="ExternalOutput").ap()

    with tile.TileContext(nc) as tc, ExitStack() as ctx:
        cpool = ctx.enter_context(tc.tile_pool(name="const", bufs=1))
        mpool = ctx.enter_context(tc.tile_pool(name="mag", bufs=1))
        vpool = ctx.enter_context(tc.tile_pool(name="v", bufs=1))
        tpool = ctx.enter_context(tc.tile_pool(name="tp", bufs=1))
        ypool = ctx.enter_context(tc.tile_pool(name="y", bufs=1))
        spool = ctx.enter_context(tc.tile_pool(name="s", bufs=1))
        upool = ctx.enter_context(tc.tile_pool(name="u", bufs=1))
        eopool = ctx.enter_context(tc.tile_pool(name="eo", bufs=1))
        papool = ctx.enter_context(tc.tile_pool(name="pa", bufs=2, space="PSUM"))
        pbpool = ctx.enter_context(tc.tile_pool(name="pb", bufs=2, space="PSUM"))

        rwt_t = []
        for kc in range(8):
            t_ = cpool.tile([128, 1024], F32, tag=f"rwt{kc}", name=f"rwt{kc}")
            nc.sync.dma_start(t_[:], rwt_d[128 * kc:128 * (kc + 1), :])
            rwt_t.append(t_)
        wet_t = []
        wot_t = []
        for kc in range(4):
            t_ = cpool.tile([128, 1024], F32, tag=f"wet{kc}", name=f"wet{kc}")
            nc.sync.dma_start(t_[:], wet_d[128 * kc:128 * (kc + 1), :])
            wet_t.append(t_)
        for kc in range(4):
            t_ = cpool.tile([128, 1024], F32, tag=f"wot{kc}", name=f"wot{kc}")
            nc.sync.dma_start(t_[:], wot_d[128 * kc:128 * (kc + 1), :])
            wot_t.append(t_)
        pt_t = cpool.tile([80, 513], F32, tag="pt", name="pt")
        nc.sync.dma_start(pt_t[:], pt_d[:])
        rwm_t = []
        for h, d in enumerate([rwm0_d, rwm1_d]):
            t_ = cpool.tile([128, 2], F32, tag=f"rwm{h}", name=f"rwm{h}")
            nc.sync.dma_start(t_[:], d[:])
            rwm_t.append(t_)
        fl_t = []
        for i in range(4):
            t_ = cpool.tile([128, 128], F32, tag=f"fl{i}", name=f"fl{i}")
            nc.sync.dma_start(t_[:], fl_d[i][:])
            fl_t.append(t_)
        bias_t = cpool.tile([128, 1], F32, tag="bias", name="bias")
        nc.vector.memset(bias_t[:], 1e-32)
        mel_t = cpool.tile([80, 1024], F32, tag="mel", name="mel")
        nc.sync.dma_start(mel_t[:], mel_d[:])

        v_t = []
        for c in range(8):
            t_ = vpool.tile([128, 1030], F32, tag=f"v{c}", name=f"v{c}")
            nc.vector.memset(t_[:, 0:3], 0.0)
            nc.vector.memset(t_[:, 1027:1030], 0.0)
            nc.sync.dma_start(t_[:, 3:1027], ang_d[128 * c:128 * (c + 1), :])
            v_t.append(t_)

        # patch/scratch tile, single partition (DVE requires equal base
        # partitions for SBUF operand pairs): column regions A/B/C scratch,
        # last region holds mag[bin 512]
        pr_t = upool.tile([1, 4096], F32, tag="patch", name="patch")
        prA = pr_t[0:1, 0:1024]
        prB = pr_t[0:1, 1024:2048]
        prC = pr_t[0:1, 2048:3072]
        prM512 = pr_t[0:1, 3072:4096]

        m_t = [mpool.tile([128, 1024], F32, tag=f"m{c}", name=f"m{c}") for c in range(4)]

        def newton_mag(dst, M, bias_ap):
            q = spool.tile([128, 1024], F32, tag="s", name="s")
            r = spool.tile([128, 1024], F32, tag="t1", name="t1")
            nc.scalar.activation(q[0:M, :], dst[0:M, :], AF.Sqrt, bias=bias_ap)
            nc.vector.reciprocal(r[0:M, :], q[0:M, :])
            nc.vector.scalar_tensor_tensor(
                dst[0:M, :], dst[0:M, :], 0.5, r[0:M, :], ALU.mult, ALU.mult)
            nc.vector.scalar_tensor_tensor(
                dst[0:M, :], q[0:M, :], 0.5, dst[0:M, :], ALU.mult, ALU.add)

        for mc in range(4):
            for half in range(2):
                ps = papool.tile([128, 512], F32, tag="a0", name="a0")
                nc.tensor.matmul(ps[:], pt_t[:, 128 * mc:128 * (mc + 1)],
                                 mel_t[:, 512 * half:512 * (half + 1)],
                                 start=True, stop=True)
                nc.vector.tensor_scalar_max(
                    m_t[mc][:, 512 * half:512 * (half + 1)], ps[:], 0.0)
            newton_mag(m_t[mc], 128, bias_t[0:128])
        # mag for bin 512 lives in the pr tile (partition 96 row)
        for half in range(2):
            ps = papool.tile([128, 512], F32, tag="a0", name="a0")
            nc.tensor.matmul(ps[0:1, :], pt_t[:, 512:513],
                             mel_t[:, 512 * half:512 * (half + 1)],
                             start=True, stop=True)
            nc.vector.tensor_scalar_max(
                prM512[:, 512 * half:512 * (half + 1)], ps[0:1, :], 0.0)
        newton_mag(prM512, 1, bias_t[0:1])

        # mag_st chunks: chunk 4 uses m0 generically; its partition-0 row
        # (bin 512) is wrong there but gets patched each iteration anyway.
        mag_st = [m_t[0], m_t[1], m_t[2], m_t[3], m_t[0], m_t[1], m_t[2], m_t[3]]

        for c in range(8):
            nc.vector.tensor_mul(v_t[c][:, 3:1027], v_t[c][:, 3:1027], mag_st[c][:])
        # fix V chunk-4 row 0: should be mag512 * ang, not mag0 * ang.
        # v4 row0 currently = m0[0,:]*ang; recompute from ang via mag512/mag0?
        # Simpler: v4row = v4row * mag512 / mag0row is ill-conditioned; instead
        # reload ang row and multiply by mag512.
        nc.sync.dma_start(prA, ang_d[512:513, :])
        nc.vector.tensor_mul(v_t[4][0:1, 3:1027], prA, prM512)

        tp_t = []
        for c in range(8):
            t_ = tpool.tile([128, 1024], F32, tag=f"tp{c}", name=f"tp{c}")
            nc.vector.memset(t_[:], 0.0)
            tp_t.append(t_)

        y_t = [ypool.tile([128, 1027], F32, tag=f"y{h}", name=f"y{h}") for h in range(2)]
        # e/o tiles per half: index = 2*a'' + h''
        # e0/e1 reuse the mel/pt const-pool slots (mel and pt are dead after
        # the spec matmuls; their slots are >= [128,512] fp32)
        e_t = [cpool.tile([128, 512], F32, tag="mel", name="e0"),
               cpool.tile([128, 512], F32, tag="pt", name="e1"),
               eopool.tile([128, 512], F32, tag="e2", name="e2"),
               eopool.tile([128, 512], F32, tag="e3", name="e3")]
        o_t = [eopool.tile([128, 512], F32, tag=f"o{i}", name=f"o{i}") for i in range(4)]

        for it in range(n_iter + 1):
            last = it == n_iter
            for h in range(2):
                pa0 = papool.tile([128, 512], F32, tag="a0", name="a0")
                pa12 = papool.tile([128, 515], F32, tag="a12", name="a12")
                for ai, a in enumerate(range(4)):
                    for ki, kc in enumerate([0, 4, 1, 5, 2, 6, 3, 7]):
                        lhsT = rwt_t[kc][:, 256 * a + 128 * h:256 * a + 128 * h + 128]
                        st = (ai == 0 and ki == 0)
                        sp = (ai == 3 and ki == 7)
                        nc.tensor.matmul(pa0[:, 0:512], lhsT,
                                         v_t[kc][:, 3 - a:515 - a],
                                         start=st, stop=sp)
                        nc.tensor.matmul(pa12[:, 0:512], lhsT,
                                         v_t[kc][:, 515 - a:1027 - a],
                                         start=st, stop=sp)
                        nc.tensor.matmul(pa12[:, 512:515], lhsT,
                                         v_t[kc][:, 1027 - a:1030 - a],
                                         start=st, stop=sp)
                # envelope: exactly 2/3 except Y cols u=2 and u=1024
                nc.vector.tensor_mul(y_t[h][:, 2:3], pa0[:, 2:3], rwm_t[h][:, 0:1])
                nc.scalar.mul(y_t[h][:, 3:512], pa0[:, 3:512], C23)
                nc.scalar.mul(y_t[h][:, 512:1024], pa12[:, 0:512], C23)
                nc.vector.tensor_mul(y_t[h][:, 1024:1025], pa12[:, 512:513],
                                     rwm_t[h][:, 1:2])
            if last:
                nc.sync.dma_start(y0_d[:], y_t[0][:, 2:1025])
                nc.sync.dma_start(y1_d[:], y_t[1][:, 2:1025])
                break

            for h in range(2):
                pe_ = pbpool.tile([128, 4], F32, tag="b", name="pe")
                for col, hh, specs in EDGE_SPECS:
                    if hh != h:
                        continue
                    for i2, (fi, sh, sc) in enumerate(specs):
                        nc.tensor.matmul(pe_[:, col:col + 1], fl_t[fi][:],
                                         y_t[sh][:, sc:sc + 1],
                                         start=(i2 == 0), stop=(i2 == len(specs) - 1))
                nc.vector.tensor_copy(y_t[h][:, 0:2], pe_[:, 0:2])
                nc.vector.tensor_copy(y_t[h][:, 1025:1027], pe_[:, 2:4])

            # ---- folded STFT, processed per 512-column half ----
            for half in range(2):
                # ZZ[h''][p, j] = Y[256-128h''-p, 2+512h+j] j=0..512 (width 513)
                #   h''=0: p>=1 <- Yh1[128-p]; p=0 <- Yh0[0, u+1]
                #   h''=1: p>=1 <- Yh0[128-p]; p=0 <- Yh1[0, u]
                u0 = 2 + 512 * half
                zz = []
                for hpp in range(2):
                    zt = papool.tile([128, 515], F32, tag="a12", name="zz")
                    src_main = y_t[1 - hpp]
                    # row p>=1: flip of the other h chunk; row 0: special source
                    # (for hpp=0 it is Y[0, u+1] whose u=1026+1 would be OOB at
                    #  the last col — that element is never consumed, skip it)
                    # start/stop per PSUM bank (zero-region) group
                    if hpp == 0:
                        mms = [
                            (fl_t[0], src_main[:, u0:u0 + 512], 0, 512, True, False),
                            (fl_t[2], y_t[0][:, u0 + 1:u0 + 513], 0, 512, False, True),
                            (fl_t[0], src_main[:, u0 + 512:u0 + 513], 512, 513, True, True),
                        ]
                    else:
                        mms = [
                            (fl_t[0], src_main[:, u0:u0 + 512], 0, 512, True, False),
                            (fl_t[2], y_t[1][:, u0:u0 + 512], 0, 512, False, True),
                            (fl_t[0], src_main[:, u0 + 512:u0 + 513], 512, 513, True, False),
                            (fl_t[2], y_t[1][:, u0 + 512:u0 + 513], 512, 513, False, True),
                        ]
                    for flt, rhs, c0, c1, st_, sp_ in mms:
                        nc.tensor.matmul(zt[:, c0:c1], flt[:], rhs,
                                         start=st_, stop=sp_)
                    zz.append(zt)
                # e/o build: e[2a''+h''][p, t'] = Y[h''][p, t'+a''+512h]
                #                                + ZZ[h''][p, 1-a''+t'-512h... ]
                for app in range(2):
                    for hpp in range(2):
                        i_ = 2 * app + hpp
                        yv = y_t[hpp][:, app + 512 * half:app + 512 * half + 512]
                        zv = zz[hpp][:, 1 - app:1 - app + 512]
                        nc.vector.tensor_tensor(e_t[i_][:], yv, zv, ALU.add)
                        nc.vector.tensor_tensor(o_t[i_][:], yv, zv, ALU.subtract)
                # row n'=0 of e chunk (a''=0,h''=0): s512 = Y[0, t'+2]
                nc.vector.tensor_copy(
                    e_t[0][0:1, :], y_t[0][0:1, 2 + 512 * half:514 + 512 * half])

                mc_order = ([0, 1, 2, 3, 4, 5, 6, 7] if half == 0
                            else [0, 4, 1, 5, 2, 6, 3, 7])
                for mc in mc_order:
                    pb = pbpool.tile([128, 512], F32, tag="b", name="pb")
                    first = True
                    if mc <= 4:
                        for kc in range(4):
                            nc.tensor.matmul(
                                pb[:], wet_t[kc][:, 128 * mc:128 * (mc + 1)],
                                e_t[kc][:], start=first,
                                stop=(mc < 4 and kc == 3))
                            first = False
                    if mc >= 4:
                        for kc in range(4):
                            nc.tensor.matmul(
                                pb[:], wot_t[kc][:, 128 * mc:128 * (mc + 1)],
                                o_t[kc][:], start=first, stop=(kc == 3))
                            first = False
                    nc.vector.scalar_tensor_tensor(
                        v_t[mc][:, 3 + 512 * half:3 + 512 * (half + 1)],
                        tp_t[mc][:, 512 * half:512 * (half + 1)],
                        -MOM, pb[:], ALU.mult, ALU.add)
                    nc.scalar.copy(tp_t[mc][:, 512 * half:512 * (half + 1)], pb[:])

                    if half == 1 and mc >= 4:
                        c = mc - 4
                        vc = v_t[c][:, 3:1027]
                        vc4 = v_t[c + 4][:, 3:1027]
                        s = spool.tile([128, 1024], F32, tag="s", name="s")
                        t1 = spool.tile([128, 1024], F32, tag="t1", name="t1")
                        nc.vector.tensor_mul(s[:], vc, vc)
                        nc.vector.tensor_mul(t1[:], vc4, vc4)
                        nc.vector.tensor_add(s[:], s[:], t1[:])
                        nc.scalar.activation(t1[:], s[:], AF.Sqrt, bias=bias_t[:])
                        nc.vector.reciprocal(s[:], t1[:])
                        nc.vector.tensor_mul(s[:], s[:], mag_st[c][:])
                        if c == 0:
                            v0r = v_t[0][0:1, 3:1027]
                            v4r = v_t[4][0:1, 3:1027]
                            nc.vector.tensor_mul(prC, v0r, v0r)
                            nc.scalar.activation(prA, prC, AF.Sqrt, bias=bias_t[0:1])
                            nc.vector.reciprocal(prC, prA)
                            nc.vector.tensor_mul(prC, prC, m_t[0][0:1, :])
                            nc.vector.tensor_mul(prA, prC, v0r)
                            nc.vector.tensor_mul(prC, v4r, v4r)
                            nc.scalar.activation(prB, prC, AF.Sqrt, bias=bias_t[0:1])
                            nc.vector.reciprocal(prC, prB)
                            nc.vector.tensor_mul(prC, prC, prM512)
                            nc.vector.tensor_mul(prB, prC, v4r)
                        nc.vector.tensor_mul(vc, vc, s[:])
                        nc.vector.tensor_mul(vc4, vc4, s[:])
                        if c == 0:
                            nc.vector.tensor_copy(v_t[0][0:1, 3:1027], prA)
                            nc.vector.tensor_copy(v_t[4][0:1, 3:1027], prB)

    nc.compile()
    _prog_cache[n_iter] = nc
    return nc


def _host_inputs(mel_spec, fb):
    consts = _constants()
    cpu = jax.devices("cpu")[0]
    with jax.default_device(cpu):
        P = np.asarray(jnp.linalg.pinv(jnp.asarray(np.asarray(fb)).T))
        k1, k2 = jax.random.split(jax.random.key(42))
        sh = (8, NF, T)
        ar = np.asarray(jax.random.uniform(k1, sh, jnp.float32))
        ai = np.asarray(jax.random.uniform(k2, sh, jnp.float32))
    pt = np.ascontiguousarray(P.T, np.float32)
    mel = np.asarray(mel_spec, np.float32)
    in_maps = []
    for b in range(8):
        ang = np.concatenate([ar[b], ai[b][1:512]], axis=0)
        in_maps.append(dict(
            mel=np.ascontiguousarray(mel[b]),
            ang=np.ascontiguousarray(ang, np.float32),
            pt=pt,
            rwt=consts["rwt"],
            wet=consts["wet"],
            wot=consts["wot"],
            rwm0=np.ascontiguousarray(consts["rwm"][0:128]),
            rwm1=np.ascontiguousarray(consts["rwm"][128:256]),
            fl0=consts["flips"][0], fl1=consts["flips"][1],
            fl2=consts["flips"][2], fl3=consts["flips"][3],
        ))
    return in_maps


def _assemble(results):
    out = np.empty((8, L - 2 * PAD), np.float32)
    for b, r in enumerate(results):
        yfull = np.concatenate([r["y0"], r["y1"]], axis=0)
        out[b] = yfull.T.reshape(-1)
    return out


def kernel(mel_spec, fb):
    nc = _build_program(N_ITER)
    in_maps = _host_inputs(mel_spec, fb)
    res = bass_utils.run_bass_kernel_spmd(nc, in_maps, core_ids=list(range(8)))
    return _assemble(res.results)
